# revision 45
# baseline (speedup 1.0000x reference)
"""Trainium2 Bass kernel v2 for BitnetFeedForward.

Same math as baseline (see kernel.py docstring), restructured for PE
occupancy:
  * Split weight-stat AllReduces: AR1 (w1 mean|.|) launches ~25us in so sw1
    is ready ~55us and M1 starts ~60us (baseline ~140us). AR2 (w2) runs
    hidden under M1; its combine ops are emitted mid-M1 to avoid DVE
    head-of-line blocking.
  * Weight ternary quant in 2 DVE ops/tile: (w*sw max -C), then
    (min C + 192) -> bf16 directly (ulp(192)=1 in bf16 gives the RNE
    integer round for free). Double-rounding vs the fp32-magic path can
    flip ~2^-17-boundary weights only (negligible vs 2e-2 gate).
  * h-quant rowsum moved to ACT (Copy bias=-MAGIC, accum_out), scale-round
    on Pool (c>0) / DVE (c==0) so DVE is free for the M2 weight stream.
  * DMA queues split by role: SP = pure input streams (shards, x, w1, w2,
    g2 readback late), ACT-dge = producer-ordered writes (xq funnels,
    transposes, g2 out, hq funnels, hqt transposes, out), Pool = collective
    plumbing only.
  * Per-m xq funnels so the xbar transposes (and M1) start earlier.

`repeat=R` (timing-only mode): emits the full computation R times in one
module, with a zero-valued carry read back from the final output DMA of
iteration i folded (as a numeric no-op max) into iteration i+1's x-absmax
stats, so iterations serialize on device. Every repetition computes the
identical (correct) output; wall-clock difference between R and 1 divides
out the multi-ms axon dispatch floor that swamps single-run timing.
"""

import numpy as np
from contextlib import ExitStack

import concourse.bass as bass
import concourse.bass_isa as bass_isa
import concourse.mybir as mybir
from concourse import tile

F32 = mybir.dt.float32
BF16 = mybir.dt.bfloat16
F8 = mybir.dt.float8e4
AX = mybir.AxisListType
OP = mybir.AluOpType
AF = mybir.ActivationFunctionType
DR = mybir.MatmulPerfMode.DoubleRow

P = 128
MAGIC = 12582912.0          # 1.5 * 2**23: fp32 round-to-nearest-even trick
WMAGIC = 192.0              # 1.5 * 2**7: bf16 RNE round for the ternary range
CLIP_B = 1.4990234375       # clamp bound making clamp+round == round+clip
EPS = 1e-5
CW = 2048                   # stats chunk width

N_CORES = 8
FULL_T, DIM, INNER = 8192, 2048, 8192
SIM_NO_GELU = False  # simtest only: CoreSim lacks Gelu; swap for Identity


def build_nc(T, D, I, n_cores=N_CORES, full_stats=False, debug=False,
             split_waits=True, repeat=1, dr1=True, dr2=True):
    n_m = T // P
    n_k1 = D // P
    n_nb1 = I // 512
    n_k2 = I // P
    n_nb2 = D // 512
    HQC = 512
    n_hc = I // HQC
    k_per_c = HQC // P
    d_shard = D if full_stats else D // n_cores
    i_shard = I if full_stats else I // n_cores
    assert n_m <= 8

    nc = bass.Bass(debug=debug)

    x_d = nc.declare_dram_parameter("x", [T, D], F32, isOutput=False)
    w1t_d = nc.declare_dram_parameter("w1t", [D, I], F32, isOutput=False)
    w2t_d = nc.declare_dram_parameter("w2t", [I, D], F32, isOutput=False)
    w1s_d = nc.declare_dram_parameter("w1s", [d_shard, I], F32, isOutput=False)
    w2s_d = nc.declare_dram_parameter("w2s", [i_shard, D], F32, isOutput=False)
    out_d = nc.declare_dram_parameter("out", [T, D], F32, isOutput=True)

    xq_d = nc.dram_tensor("xq_scr", [T, D], BF16)
    g2_d = nc.dram_tensor("g2_scr", [T, I], BF16)
    hq_d = nc.dram_tensor("hq_scr", [T, I], BF16)
    scr2 = nc.dram_tensor("scr2", [P, 1], F32)
    scr2b = nc.dram_tensor("scr2b", [1, P], F32)
    if not full_stats:
        cc1_in = nc.dram_tensor("cc1_in", [P, 1], F32)
        cc1_out = nc.dram_tensor("cc1_out", [P, 1], F32, addr_space="Shared")
        cc2_in = nc.dram_tensor("cc2_in", [1, P], F32)
        cc2_out = nc.dram_tensor("cc2_out", [1, P], F32, addr_space="Shared")

    with tile.TileContext(nc) as tc, ExitStack() as ctx:
        persist = ctx.enter_context(tc.tile_pool(name="persist", bufs=1))

        def pt(shape, dtype, tag):
            return persist.tile(shape, dtype, tag=tag, name=tag)

        junk = pt([P, 512], F32, "junk")
        ss_x = pt([P, n_m], F32, "ss_x")
        am_x = pt([P, n_m], F32, "am_x")
        rs_x = pt([P, n_m], F32, "rs_x")
        s_x = pt([P, n_m], F32, "s_x")
        cm1p = pt([P, n_m], F32, "cm1p")
        cm1 = pt([P, n_m], F32, "cm1")
        bias1 = pt([P, n_m], F32, "bias1")
        ssh_p = pt([P, n_m * n_nb1], F32, "ssh_p")
        amh_p = pt([P, n_m * n_nb1], F32, "amh_p")
        rsh_p = pt([P, n_m * n_hc], F32, "rsh_p")
        ss_h = pt([P, n_m], F32, "ss_h")
        am_h = pt([P, n_m], F32, "am_h")
        rs_h = pt([P, n_m], F32, "rs_h")
        s_h = pt([P, n_m], F32, "s_h")
        cm2p = pt([P, n_m], F32, "cm2p")
        cm2 = pt([P, n_m], F32, "cm2")
        bias2 = pt([P, n_m], F32, "bias2")
        wsum = pt([P, 2], F32, "wsum")
        dw = pt([P, 2], F32, "dw")
        sw = pt([P, 2], F32, "sw")
        sc1 = pt([P, 1], F32, "sc1")
        sc2 = pt([P, 1], F32, "sc2")
        tmp1 = pt([P, 1], F32, "tmp1")
        tmp2 = pt([P, 1], F32, "tmp2")
        t8a = pt([P, n_m], F32, "t8a")
        t8b = pt([P, n_m], F32, "t8b")
        t8c = pt([P, n_m], F32, "t8c")
        y0 = pt([P, n_m], F32, "y0")
        y1 = pt([P, n_m], F32, "y1")
        ones = pt([P, P], F32, "ones")
        carry = pt([P, 1], F32, "carry")
        carry_sb = pt([P, 1], F32, "carry_sb")

        v = nc.vector
        act = nc.scalar

        v.memset(ones[:], 1.0)

        def scalar_chain_pre(ss, am, s_out, pre_out, d, sl=slice(None)):
            """s_out = 127/absmax; pre_out = am*sqrt(d)/(127*max(n,1e-12)).
            (Final dequant c = pre_out * dw applied separately once dw is
            known.) `sl` selects token-block columns so per-block chains can
            run as soon as that block's stats land."""
            v.tensor_scalar(t8a[:, sl], am[:, sl], 1e-20, None, OP.max)
            v.reciprocal(t8b[:, sl], t8a[:, sl])
            v.tensor_scalar(s_out[:, sl], t8b[:, sl], 127.0, None, OP.mult)
            act.sqrt(y0[:, sl], ss[:, sl])
            v.tensor_scalar(y1[:, sl], y0[:, sl], 1e-20, None, OP.max)
            v.reciprocal(t8b[:, sl], y1[:, sl])
            v.tensor_mul(t8a[:, sl], ss[:, sl], t8b[:, sl])          # ss/y
            v.tensor_add(t8c[:, sl], t8a[:, sl], y1[:, sl])          # y + ss/y
            v.tensor_scalar(y1[:, sl], t8c[:, sl], 0.5, 1e-12,
                            OP.mult, OP.max)                          # n
            v.reciprocal(t8b[:, sl], y1[:, sl])                      # 1/n
            v.tensor_mul(t8a[:, sl], am[:, sl], t8b[:, sl])          # am/n
            v.tensor_scalar(pre_out[:, sl], t8a[:, sl],
                            float(np.sqrt(d) / 127.0), None, OP.mult)

        for rep in range(repeat):
            emit_iteration(
                nc, tc, rep, repeat, locals())

    if split_waits:
        _split_waits(nc)
    return nc


def emit_iteration(nc, tc, rep, repeat, env):
    """One full forward pass. `env` carries build_nc's locals (persist
    tiles, dims, dram handles, helpers)."""
    g = env
    T, D, I = g["T"], g["D"], g["I"]
    n_m, n_k1, n_nb1, n_k2, n_nb2 = (g["n_m"], g["n_k1"], g["n_nb1"],
                                     g["n_k2"], g["n_nb2"])
    HQC, n_hc, k_per_c = g["HQC"], g["n_hc"], g["k_per_c"]
    d_shard, i_shard = g["d_shard"], g["i_shard"]
    full_stats, n_cores = g["full_stats"], g["n_cores"]
    x_d, w1t_d, w2t_d, w1s_d, w2s_d, out_d = (g["x_d"], g["w1t_d"],
                                              g["w2t_d"], g["w1s_d"],
                                              g["w2s_d"], g["out_d"])
    xq_d, g2_d, hq_d, scr2, scr2b = (g["xq_d"], g["g2_d"], g["hq_d"],
                                     g["scr2"], g["scr2b"])
    if not full_stats:
        cc1_in, cc1_out = g["cc1_in"], g["cc1_out"]
        cc2_in, cc2_out = g["cc2_in"], g["cc2_out"]
    pt, scalar_chain_pre = g["pt"], g["scalar_chain_pre"]
    v, act = g["v"], g["act"]
    dr1, dr2 = g["dr1"], g["dr2"]
    (junk, ss_x, am_x, rs_x, s_x, cm1p, cm1, bias1, ssh_p, amh_p, rsh_p,
     ss_h, am_h, rs_h, s_h, cm2p, cm2, bias2, wsum, dw, sw, sc1, sc2,
     tmp1, tmp2, t8a, t8b, t8c, y0, y1, ones, carry, carry_sb) = (
        g["junk"], g["ss_x"], g["am_x"], g["rs_x"], g["s_x"], g["cm1p"],
        g["cm1"], g["bias1"], g["ssh_p"], g["amh_p"], g["rsh_p"], g["ss_h"],
        g["am_h"], g["rs_h"], g["s_h"], g["cm2p"], g["cm2"], g["bias2"],
        g["wsum"], g["dw"], g["sw"], g["sc1"], g["sc2"], g["tmp1"],
        g["tmp2"], g["t8a"], g["t8b"], g["t8c"], g["y0"], g["y1"],
        g["ones"], g["carry"], g["carry_sb"])
    P_ = P

    # Pool open/close must be LIFO (stack allocator): open the
    # longer-lived pools first, then the startup-only xph pool last so it
    # can close first and its ~120KB/partition is reusable by hqt.
    xqt_ctx = ExitStack()
    xqt_pool = xqt_ctx.enter_context(tc.tile_pool(name="xqt", bufs=1))
    wsp_ctx = ExitStack()
    wsp = wsp_ctx.enter_context(tc.tile_pool(name="wstat", bufs=3))
    m1w_ctx = ExitStack()
    m1w = m1w_ctx.enter_context(tc.tile_pool(name="m1w", bufs=3))
    m1g = m1w_ctx.enter_context(tc.tile_pool(name="m1g", bufs=3))
    xph_ctx = ExitStack()
    xph = xph_ctx.enter_context(tc.tile_pool(name="xph", bufs=1))
    junk_big = xph.tile([P, CW], F32, tag="junk_big", name="junk_big")

    # ---- Phase S1: w1 mean|.| shard stats -> AR1 (launched ASAP) ----

    def wstats_emit(src_d, rows_total, width, parts_tag, jt, cw,
                    dma_wait=None):
        n_t = (rows_total + P - 1) // P
        n_ch = (width + cw - 1) // cw
        n_acc = cw // jt.shape[1]
        parts = pt([P, n_t * n_ch * n_acc], F32, parts_tag)
        v.memset(parts[:], 0.0)
        for i in range(n_t):
            rows = min(P, rows_total - i * P)
            for j in range(n_ch):
                w = min(cw, width - j * cw)
                wt = wsp.tile([P, cw], F32, tag=f"ws{cw}", name="ws")
                with tc.tile_wait_until(dma_wait or 0,
                                        enable=dma_wait is not None):
                    nc.sync.dma_start(wt[:rows, :w],
                                      src_d[i * P:i * P + rows,
                                            j * cw:j * cw + w])
                for a in range(0, w, jt.shape[1]):
                    aw = min(jt.shape[1], w - a)
                    col = (i * n_ch + j) * n_acc + a // jt.shape[1]
                    act.activation(jt[:rows, :aw], wt[:rows, a:a + aw],
                                   AF.Abs,
                                   accum_out=parts[:rows, col:col + 1])
        return parts

    parts1 = wstats_emit(w1s_d, d_shard, I, "parts1", junk_big, CW)
    with tc.high_priority():
        v.tensor_reduce(tmp1[:], parts1[:], axis=AX.X, op=OP.add)
        with tc.tile_pool(name="wstps", bufs=1, space="PSUM") as wsps:
            psb = wsps.tile([P, 1], F32, tag="psb", name="psb")
            nc.tensor.matmul(psb[:], ones[:], tmp1[:], start=True,
                             stop=True)
            v.tensor_copy(wsum[:, 0:1], psb[:])
        if not full_stats:
            # tiny-collective plumbing rides the Pool queue (never
            # contended this early); SP stays free for the input streams.
            nc.gpsimd.dma_start(cc1_in[:, :], wsum[:, 0:1])
            nc.gpsimd.collective_compute(
                "AllReduce", OP.add,
                replica_groups=[list(range(n_cores))],
                ins=[cc1_in.ap().opt()], outs=[cc1_out.ap().opt()],
            )
        else:
            v.tensor_copy(sc1[:], wsum[:, 0:1])

    # ---- Phase X: token stats + int-grid quantization of x ----
    # x loads are scheduler-delayed to ~24us (after the w1s shard stream)
    # so the w1-stats -> AR1 chain owns the DMA pipe first; the per-block
    # stat/chain/quant/funnel pipeline then chases the x stream and
    # finishes ~2us after the last x tile lands.
    xq_big = xph.tile([P, n_m * D], BF16, tag="xq_big", name="xq_big")
    xqt_t = [xqt_pool.tile([P, T], BF16, tag=f"xqt{k}", name=f"xqt{k}")
             for k in range(n_k1)]
    for m in range(n_m):
        xt = xph.tile([P, D], F32, tag="xt", name="xt", bufs=3)
        # stagger the late x tiles so the tiny AR1 input transfer gets a
        # DMA-pipe slot right after the w1 stats instead of ~16us later
        with tc.tile_wait_until(0.012 if m < 4 else 0.026, enable=(rep == 0)):
            nc.scalar.dma_start(xt[:], x_d[m * P:(m + 1) * P, :])
        act.activation(junk_big[:, :D], xt[:], AF.Square,
                       accum_out=ss_x[:, m:m + 1])
        v.tensor_reduce(am_x[:, m:m + 1], xt[:], axis=AX.X,
                        op=OP.max, apply_absolute_value=True)
        if rep > 0:
            # zero-valued carry from iteration rep-1's final output DMA:
            # numeric no-op, serializes iterations in repeat/timing mode.
            v.tensor_scalar(am_x[:, m:m + 1], am_x[:, m:m + 1],
                            carry[:, 0:1], None, OP.max)
        scalar_chain_pre(ss_x, am_x, s_x, cm1p, D, sl=slice(m, m + 1))
        tq = xph.tile([P, D], F32, tag="tqx", name="tqx", bufs=3)
        if m % 2 == 0:
            v.tensor_scalar(tq[:], xt[:], s_x[:, m:m + 1], MAGIC,
                            OP.mult, OP.add)
            v.tensor_scalar(xq_big[:, m * D:(m + 1) * D], tq[:], MAGIC,
                            None, OP.subtract, OP.add,
                            accum_out=rs_x[:, m:m + 1])
        else:
            nc.gpsimd.tensor_scalar(tq[:], xt[:], s_x[:, m:m + 1], MAGIC,
                                    OP.mult, OP.add)
            act.activation(xq_big[:, m * D:(m + 1) * D], tq[:], AF.Copy,
                           bias=-MAGIC, scale=1.0,
                           accum_out=rs_x[:, m:m + 1])
        # per-m SBUF->SBUF 128x128 xbar transposes: no DRAM roundtrip, and
        # M1 k=0 only needs block m's slice, so the PE can start as soon as
        # sw1 and the early m-blocks are ready.
        for k in range(n_k1):
            nc.sync.dma_start(
                xqt_t[k][:, m * P:(m + 1) * P],
                xq_big[:, m * D + k * P:m * D + (k + 1) * P],
                transpose=True)

    if dr1:
        # fp8 (coarse, residual) DoubleRow pair split per k-tile:
        #   xc = fp8(xq); out = xc.T @ w + res.T @ w == xq.T @ w exactly.
        xqp = []
        for k in range(n_k1):
            p_ = xqt_pool.tile([P, 2 * T], F8, tag=f"xqp{k}", name=f"xqp{k}")
            v.tensor_scalar(p_[:, 0:T], xqt_t[k][:], 1.0, None, OP.mult)
            v.tensor_tensor(p_[:, T:2 * T], xqt_t[k][:], p_[:, 0:T],
                            OP.subtract)
            xqp.append(p_)
    else:
        xqp = xqt_t
    xph_ctx.close()

    # sw1 / cm1 / bias1: emitted after the X-phase DVE ops so the wait on
    # AR1 does not head-of-line-block the x quantization. The sc1
    # readback is likewise emitted here: on the Pool queue it must not
    # sit ahead of the x-quant Pool ops while the collective runs.
    if not full_stats:
        nc.gpsimd.dma_start(sc1[:], cc1_out[:, :])
    v.tensor_scalar(dw[:, 0:1], sc1[:], 1.0 / (D * I), EPS,
                    OP.mult, OP.max)
    v.reciprocal(sw[:, 0:1], dw[:, 0:1])
    v.tensor_scalar(cm1[:], cm1p[:], dw[:, 0:1], None, OP.mult)
    if not dr1:
        # bf16 weights carry a +WMAGIC offset; corrected via the gelu bias
        v.tensor_mul(t8c[:], cm1[:], rs_x[:])
        v.tensor_scalar(bias1[:], t8c[:], -WMAGIC, None, OP.mult)

    # ---- Phase S2 emit helper (runs hidden under M1 nb0) ----
    t2row = pt([1, P], F32, "t2row")
    t2rep = pt([1, P], F32, "t2rep")

    def emit_w2_stats_part(part, n_parts):
        # one slice of the w2 shard's abs-accumulate, so the ACT work is
        # spread over several M1 nb blocks instead of clogging nb0
        rows = i_shard // n_parts
        return wstats_emit(w2s_d[part * rows:(part + 1) * rows, :], rows,
                           D, f"parts2_{part}", junk, CW, dma_wait=0.1)

    def emit_w2_stats_tail(parts_list):
        # Cross-partition sum + broadcast without the PE (which is busy
        # with M1): bounce the per-partition partials through DRAM into
        # one partition row, reduce+replicate on DVE, AllReduce the
        # replicated row, then scatter it back across partitions.
        for i, p2 in enumerate(parts_list):
            v.tensor_reduce(tmp1[:] if False else t8a[:, 0:1], p2[:],
                            axis=AX.X, op=OP.add)
            if i == 0:
                v.tensor_copy(tmp2[:], t8a[:, 0:1])
            else:
                v.tensor_add(tmp2[:], tmp2[:], t8a[:, 0:1])
        nc.gpsimd.dma_start(scr2[:, :], tmp2[:])
        nc.gpsimd.dma_start(
            t2row[:], scr2[:, :].rearrange("p one -> one p"))
        v.tensor_reduce(t2rep[0:1, 0:1], t2row[:], axis=AX.X, op=OP.add)
        v.tensor_scalar(t2rep[:], t2row[:], 0.0, t2rep[0:1, 0:1],
                        OP.mult, OP.add)
        if not full_stats:
            nc.gpsimd.dma_start(cc2_in[:, :], t2rep[:])
            nc.gpsimd.collective_compute(
                "AllReduce", OP.add,
                replica_groups=[list(range(n_cores))],
                ins=[cc2_in.ap().opt()], outs=[cc2_out.ap().opt()],
            )
        else:
            nc.gpsimd.dma_start(scr2b[:, :], t2rep[:])

    def emit_sc2_read():
        # deferred to nb==6: the collective has completed by then, so this
        # readback never head-of-line-blocks the Pool queue mid-M1.
        if not full_stats:
            nc.gpsimd.dma_start(
                sc2[:], cc2_out[:, :].rearrange("one p -> p one"))
        else:
            nc.gpsimd.dma_start(
                sc2[:], scr2b[:, :].rearrange("one p -> p one"))

    def emit_sw2():
        v.tensor_scalar(dw[:, 1:2], sc2[:], 1.0 / (D * I), EPS,
                        OP.mult, OP.max)
        v.reciprocal(sw[:, 1:2], dw[:, 1:2])

    w2_parts = []
    # ---- Phase M1: h = xq @ w1q'^T, fused ternary quant of w1 ----
    with tc.tile_pool(name="m1ps", bufs=1, space="PSUM") as m1ps:
        for nb in range(n_nb1):
            ps = [m1ps.tile([P, 512], F32, tag=f"ps{m}", name=f"ps{m}")
                  for m in range(n_m)]
            for k in range(n_k1):
                wf = m1w.tile([P, 512], F32, tag="wf", name="wf", bufs=6)
                nc.sync.dma_start(wf[:], w1t_d[k * P:(k + 1) * P,
                                               nb * 512:(nb + 1) * 512])
                tf = m1w.tile([P, 512], F32, tag="tf", name="tf", bufs=6)
                tg = m1w.tile([P, 512], F32, tag="tg", name="tg", bufs=6)
                wq = m1w.tile([P, 512], F8 if dr1 else BF16, tag="wq",
                              name="wq", bufs=8)
                # scheduler-delay nb0's quant ops past the x phase: the
                # scheduler's fast model thinks sw1 is ready early and
                # would order these ahead of the x quants on DVE, where
                # the runtime wait on AR1 then head-of-line-blocks the
                # whole x pipeline.
                with tc.tile_wait_until(0.082, enable=(nb == 0 and rep == 0)):
                    v.tensor_scalar(tf[:], wf[:], sw[:, 0:1], -CLIP_B,
                                    OP.mult, OP.max)
                    v.tensor_scalar(tg[:], tf[:], CLIP_B, MAGIC,
                                    OP.min, OP.add)
                    v.tensor_scalar(wq[:], tg[:],
                                    MAGIC if dr1 else MAGIC - WMAGIC, None,
                                    OP.subtract)
                if dr1:
                    wqb = wq[:].unsqueeze(1).broadcast_to([P, 2, 512])
                    pv = xqp[k][:].rearrange("p (two m) -> p two m", two=2)
                for m in range(n_m):
                    if dr1:
                        nc.tensor.matmul(ps[m][:],
                                         pv[:, :, m * P:(m + 1) * P],
                                         wqb, start=(k == 0),
                                         stop=(k == n_k1 - 1), perf_mode=DR)
                    else:
                        nc.tensor.matmul(ps[m][:],
                                         xqp[k][:, m * P:(m + 1) * P],
                                         wq[:], start=(k == 0),
                                         stop=(k == n_k1 - 1))
            for m in range(n_m):
                idx = m * n_nb1 + nb
                gf = AF.Identity if SIM_NO_GELU else AF.Gelu
                g1 = m1g.tile([P, 512], F32, tag="g1", name="g1")
                if dr1:
                    act.activation(g1[:], ps[m][:], gf,
                                   scale=cm1[:, m:m + 1])
                else:
                    act.activation(g1[:], ps[m][:], gf,
                                   bias=bias1[:, m:m + 1],
                                   scale=cm1[:, m:m + 1])
                g2 = m1g.tile([P, 512], BF16, tag="g2", name="g2")
                act.activation(g2[:], g1[:], gf)
                act.activation(junk[:, :512], g2[:], AF.Square,
                               accum_out=ssh_p[:, idx:idx + 1])
                v.tensor_reduce(amh_p[:, idx:idx + 1], g2[:], axis=AX.X,
                                op=OP.max, apply_absolute_value=True)
                nc.sync.dma_start(g2_d[m * P:(m + 1) * P,
                                        nb * 512:(nb + 1) * 512], g2[:])
            if nb < 4:
                w2_parts.append(emit_w2_stats_part(nb, 4))
            if nb == 4:
                emit_w2_stats_tail(w2_parts)
            if nb == 6:
                emit_sc2_read()
                emit_sw2()
    m1w_ctx.close()
    wsp_ctx.close()
    xqt_ctx.close()

    ictx = ExitStack()
    m2w = ictx.enter_context(tc.tile_pool(name="m2w", bufs=3))
    # Prefetch the first 4 wq2 quant chains: they depend only on sw2
    # (ready ~nb6) + the w2 stream, so at the layer boundary DVE is free
    # for the HS chain and chunk-0 h-quant and the PE's first M2 k-tiles
    # have weights waiting.
    wq2_pre = {}
    for k in range(4):
        wf = m2w.tile([P, 512], F32, tag="wf2", name="wf2", bufs=5)
        nc.sync.dma_start(wf[:], w2t_d[k * P:(k + 1) * P, 0:512])
        tf = m2w.tile([P, 512], F32, tag="tf2", name="tf2", bufs=4)
        v.tensor_scalar(tf[:], wf[:], sw[:, 1:2], -CLIP_B,
                        OP.mult, OP.max)
        tg = m2w.tile([P, 512], F32, tag="tg2", name="tg2", bufs=3)
        v.tensor_scalar(tg[:], tf[:], CLIP_B, MAGIC, OP.min, OP.add)
        wq = m2w.tile([P, 512], F8 if dr2 else BF16, tag="wq2", name="wq2",
                      bufs=5)
        v.tensor_scalar(wq[:], tg[:], MAGIC if dr2 else MAGIC - WMAGIC,
                        None, OP.subtract)
        wq2_pre[k] = wq

    # ---- Phase HS: h-layer per-token scales ----
    v.tensor_reduce(ss_h[:], ssh_p[:].rearrange("p (m b) -> p m b", b=n_nb1),
                    axis=AX.X, op=OP.add)
    v.tensor_reduce(am_h[:], amh_p[:].rearrange("p (m b) -> p m b", b=n_nb1),
                    axis=AX.X, op=OP.max)
    scalar_chain_pre(ss_h, am_h, s_h, cm2p, I)
    v.tensor_scalar(cm2[:], cm2p[:], dw[:, 1:2], None, OP.mult)

    # ---- Phase HQ + M2, interleaved emission ----
    # HQ chunk c produces hqt[4c..4c+3]; M2 consumes them k-ascending.
    # Chunks 0-1 are emitted before M2; chunk c>=2 is emitted inside M2
    # nb0's k-loop right after the k=4(c-2)+3 weight tile, so on every
    # queue (SP: g2 readback between w2 tiles; DVE/Pool: scale-round
    # between wq2 quants; ACT: rowsum/funnel/transpose between evacs)
    # instructions stay in data-ready order and nothing head-of-line
    # blocks the M2 weight stream.
    hqt_pool = ictx.enter_context(tc.tile_pool(name="hqt", bufs=1))
    if dr2:
        hqp = [hqt_pool.tile([P, 2 * T], F8, tag=f"hqp{k}", name=f"hqp{k}")
               for k in range(n_k2)]
        hqr_pool = ictx.enter_context(tc.tile_pool(name="hqr", bufs=4))
    else:
        hqp = [hqt_pool.tile([P, T], BF16, tag=f"hqt{k}", name=f"hqt{k}")
               for k in range(n_k2)]
    gtp = ictx.enter_context(tc.tile_pool(name="gtp", bufs=3))
    hp = ictx.enter_context(tc.tile_pool(name="hqp", bufs=1))
    gts = {}

    def emit_g2r(c):
        gt = gtp.tile([P, n_m * HQC], BF16, tag="gt", name="gt")
        nc.sync.dma_start(
            gt[:].rearrange("p (m d) -> p m d", d=HQC),
            g2_d[:, c * HQC:(c + 1) * HQC].rearrange("(m p) d -> p m d",
                                                     p=P))
        gts[c] = gt

    def emit_hq_chunk(c):
        # even chunks: scale-round AND round-sub on DVE; odd chunks:
        # scale-round on Pool, round-sub on ACT -- balances the three
        # engines against the concurrent wq2 quant stream (DVE), evacs
        # (ACT) and keeps the gt-buffer release pace steady.
        hq_big = hp.tile([P, n_m * HQC], BF16, tag="hq_big",
                         name="hq_big", bufs=1)
        gt = gts.pop(c)
        for m in range(n_m):
            tq = hp.tile([P, HQC], F32, tag="tqh", name="tqh", bufs=2)
            idx = m * n_hc + c
            sl = slice(m * HQC, (m + 1) * HQC)
            gsl = gt[:, m * HQC:(m + 1) * HQC]
            # alternate the scale-round between DVE and Pool, and the
            # round-sub between ACT and DVE, so no engine paces the chunk
            # pipeline alone (it must match the PE's k-consumption rate)
            eng = v if (c + m) % 2 == 0 else nc.gpsimd
            eng.tensor_scalar(tq[:], gsl, s_h[:, m:m + 1],
                              MAGIC, OP.mult, OP.add)
            if m % 2 == 0:
                act.activation(hq_big[:, sl], tq[:], AF.Copy, bias=-MAGIC,
                               scale=1.0, accum_out=rsh_p[:, idx:idx + 1])
            else:
                v.tensor_scalar(hq_big[:, sl], tq[:], MAGIC, None,
                                OP.subtract, OP.add,
                                accum_out=rsh_p[:, idx:idx + 1])
        # whole-chunk funnel + transposes on SP: keeps the 2.9us funnel
        # transfers and their waits off the ACT queue, which carries the
        # wf2 weight triggers + h-quant rowsums and was starving the M2
        # weight stream at the nb0 tail.
        nc.gpsimd.dma_start(
            hq_d[:, c * HQC:(c + 1) * HQC].rearrange("(m p) d -> p m d", p=P),
            hq_big[:].rearrange("p (m d) -> p m d", d=HQC))
        for kk in range(k_per_c):
            k = c * k_per_c + kk
            if dr2:
                ht = hqr_pool.tile([P, T], BF16, tag="hqtr", name="hqtr")
                nc.sync.dma_start(ht[:], hq_d[:, k * P:(k + 1) * P],
                                  transpose=True)
                # fp8 (coarse, residual) pair split for DoubleRow
                v.tensor_scalar(hqp[k][:, 0:T], ht[:], 1.0, None, OP.mult)
                v.tensor_tensor(hqp[k][:, T:2 * T], ht[:], hqp[k][:, 0:T],
                                OP.subtract)
            else:
                nc.sync.dma_start(hqp[k][:], hq_d[:, k * P:(k + 1) * P],
                                  transpose=True)

    emit_g2r(0)
    emit_g2r(1)
    emit_hq_chunk(0)
    emit_hq_chunk(1)

    with (
        tc.tile_pool(name="m2o", bufs=3) as m2o,
        tc.tile_pool(name="m2ps", bufs=1, space="PSUM") as m2ps,
    ):
        for nb in range(n_nb2):
            ps = [m2ps.tile([P, 512], F32, tag=f"ps{m}", name=f"ps{m}")
                  for m in range(n_m)]
            for k in range(n_k2):
                if nb == 0 and k < 4:
                    wq = wq2_pre[k]
                    if k == k_per_c - 1 and 2 < n_hc:
                        emit_g2r(2)
                        emit_hq_chunk(2)
                    for m in range(n_m):
                        if dr2:
                            wqb = wq[:].unsqueeze(1).broadcast_to([P, 2, 512])
                            pv = hqp[k][:].rearrange("p (two m) -> p two m",
                                                     two=2)
                            nc.tensor.matmul(ps[m][:],
                                             pv[:, :, m * P:(m + 1) * P],
                                             wqb, start=(k == 0),
                                             stop=(k == n_k2 - 1),
                                             perf_mode=DR)
                        else:
                            nc.tensor.matmul(ps[m][:],
                                             hqp[k][:, m * P:(m + 1) * P],
                                             wq[:], start=(k == 0),
                                             stop=(k == n_k2 - 1))
                    continue
                wf = m2w.tile([P, 512], F32, tag="wf2", name="wf2", bufs=5)
                nc.gpsimd.dma_start(wf[:], w2t_d[k * P:(k + 1) * P,
                                                 nb * 512:(nb + 1) * 512])
                tf = m2w.tile([P, 512], F32, tag="tf2", name="tf2", bufs=4)
                v.tensor_scalar(tf[:], wf[:], sw[:, 1:2], -CLIP_B,
                                OP.mult, OP.max)
                tg = m2w.tile([P, 512], F32, tag="tg2", name="tg2", bufs=3)
                v.tensor_scalar(tg[:], tf[:], CLIP_B, MAGIC,
                                OP.min, OP.add)
                wq = m2w.tile([P, 512], F8 if dr2 else BF16, tag="wq2",
                              name="wq2", bufs=5)
                v.tensor_scalar(wq[:], tg[:],
                                MAGIC if dr2 else MAGIC - WMAGIC,
                                None, OP.subtract)
                if nb == 0 and k % k_per_c == k_per_c - 1:
                    c = k // k_per_c + 2
                    if c < n_hc:
                        emit_g2r(c)
                        emit_hq_chunk(c)
                if dr2:
                    wqb = wq[:].unsqueeze(1).broadcast_to([P, 2, 512])
                    pv = hqp[k][:].rearrange("p (two m) -> p two m", two=2)
                for m in range(n_m):
                    if dr2:
                        nc.tensor.matmul(ps[m][:],
                                         pv[:, :, m * P:(m + 1) * P],
                                         wqb, start=(k == 0),
                                         stop=(k == n_k2 - 1), perf_mode=DR)
                    else:
                        nc.tensor.matmul(ps[m][:],
                                         hqp[k][:, m * P:(m + 1) * P],
                                         wq[:], start=(k == 0),
                                         stop=(k == n_k2 - 1))
            if nb == 0 and not dr2:
                # rs_h complete once the last HQ chunk's ACT op ran; the
                # bf16 +WMAGIC weight offset is corrected via the evac bias.
                v.tensor_reduce(rs_h[:],
                                rsh_p[:].rearrange("p (m b) -> p m b",
                                                   b=n_hc),
                                axis=AX.X, op=OP.add)
                v.tensor_mul(t8a[:], cm2[:], rs_h[:])
                v.tensor_scalar(bias2[:], t8a[:], -WMAGIC, None, OP.mult)
            for m in range(n_m):
                o = m2o.tile([P, 512], F32, tag="o", name="o")
                if dr2:
                    act.activation(o[:], ps[m][:], AF.Identity,
                                   scale=cm2[:, m:m + 1])
                else:
                    act.activation(o[:], ps[m][:], AF.Identity,
                                   bias=bias2[:, m:m + 1],
                                   scale=cm2[:, m:m + 1])
                nc.sync.dma_start(out_d[m * P:(m + 1) * P,
                                        nb * 512:(nb + 1) * 512], o[:])
    ictx.close()

    if repeat > 1:
        # serializer for timing mode: read back one column of the final
        # output tile (RAW on the last out DMA), scale to exactly zero.
        nc.sync.dma_start(carry_sb[:, 0:1], out_d[T - P:T, D - 1:D])
        v.tensor_scalar(carry[:, 0:1], carry_sb[:, 0:1], 0.0, None,
                        OP.mult)


# walrus TPB-instruction encodings accept only ONE sem-wait condition on this
# compile path; move extra waits onto standalone EventSemaphore instructions
# just before the instruction on the same engine queue.
_WAIT_OK = {"InstEventSemaphore"}


def _split_waits(nc, limit=1):
    cnt = 0
    for fn in nc.m.functions:
        for bb in fn.blocks:
            out = []
            for ins in bb.instructions:
                si = ins.sync_info
                waits = list(si.on_wait) if (si and si.on_wait) else []
                if type(ins).__name__ not in _WAIT_OK and len(waits) > limit:
                    extra, keep = waits[:-limit], waits[-limit:]
                    for w in extra:
                        cnt += 1
                        out.append(mybir.InstEventSemaphore(
                            name=f"WSPLIT-{cnt}-{ins.name}", engine=ins.engine,
                            sync_info=mybir.SyncInfo(on_wait=[w], on_update=[])))
                    try:
                        si.on_wait = keep
                    except Exception:
                        ins.sync_info = mybir.SyncInfo(on_wait=keep,
                                                       on_update=si.on_update)
                out.append(ins)
            bb.instructions[:] = out
    return cnt


_NC_CACHE = {}


def _get_nc(key, **kw):
    if key not in _NC_CACHE:
        _NC_CACHE[key] = build_nc(**kw)
    return _NC_CACHE[key]


def make_in_maps(xf, w1t, w2t, T, n_cores, full_stats=False):
    D, I = w1t.shape
    ds = D if full_stats else D // n_cores
    ish = I if full_stats else I // n_cores
    maps = []
    for c in range(n_cores):
        maps.append({
            "x": np.ascontiguousarray(xf[c * T:(c + 1) * T]),
            "w1t": w1t,
            "w2t": w2t,
            "w1s": w1t if full_stats else np.ascontiguousarray(w1t[c * ds:(c + 1) * ds]),
            "w2s": w2t if full_stats else np.ascontiguousarray(w2t[c * ish:(c + 1) * ish]),
        })
    return maps


def kernel(x, w1, w2):
    from concourse.bass_utils import run_bass_kernel_spmd

    x = np.asarray(x, dtype=np.float32)
    w1 = np.asarray(w1, dtype=np.float32)
    w2 = np.asarray(w2, dtype=np.float32)
    b, s, d = x.shape
    T = (b * s) // N_CORES
    xf = np.ascontiguousarray(x.reshape(b * s, d))
    w1t = np.ascontiguousarray(w1.T)
    w2t = np.ascontiguousarray(w2.T)

    nc = _get_nc("main", T=T, D=DIM, I=INNER, n_cores=N_CORES, full_stats=False)
    in_maps = make_in_maps(xf, w1t, w2t, T, N_CORES, full_stats=False)
    res = run_bass_kernel_spmd(nc, in_maps, list(range(N_CORES)))
    outs = [res.results[c]["out"] for c in range(N_CORES)]
    return np.concatenate(outs, axis=0).reshape(b, s, d).astype(np.float32)
